# revision 1
# baseline (speedup 1.0000x reference)
"""Trainium2 Bass kernel for nn_CopyGenerator (scatter_memory).

Computation (see the reference):
  out_tgt = log_softmax(hidden @ W.T + b)                    [T,B,VT]
  gate1m  = 1 - sigmoid(dec @ Wc.T + bc)                     [T,B]
  ext[t,b,v] = gate1m[t,b] * sum_s attn[t,b,s]*(idx[s,b]==v), idx==UNK masked
  out_ext = log(clip(ext, 1e-3, 1-1e-3))                     [T,B,VE]
  out = concat([out_tgt, out_ext], -1)

Sharding (8 cores):
  - Big matmul + log_softmax: column-parallel over tgt vocab (each core owns a
    4000-wide W slice, SBUF-resident bf16; all 3200 rows). The softmax
    denominator needs the full-vocab sum -> per-chunk partial row sums are
    AllReduced across cores (5 tiny pipelined collectives).
  - Copy-gate + scatter-add over ext vocab: data-parallel over batch (8 batch
    elements per core). The scatter-add is aw.T @ onehot(idx) on the tensor
    engine (exact, handles duplicate indices); attn is fed as a bf16 hi/lo
    pair so the ext sums are fp32-accurate.
"""

import sys

if "/opt/trn_rl_repo" not in sys.path:
    sys.path.insert(0, "/opt/trn_rl_repo")

from contextlib import ExitStack

import ml_dtypes
import numpy as np

import concourse.bass as bass
import concourse.mybir as mybir
import concourse.tile as tile
from concourse import bacc
from concourse.bass_utils import run_bass_kernel_spmd

F32 = mybir.dt.float32
BF16 = mybir.dt.bfloat16
AF = mybir.ActivationFunctionType
OP = mybir.AluOpType

T, B, S, H = 50, 64, 100, 512
VT, VE = 32000, 5000
N_CORES = 8
VL = VT // N_CORES       # 4000 vocab cols per core
BL = B // N_CORES        # 8 batch per core (ext part)
R = T * B                # 3200 rows
RL = T * BL              # 400 rows (ext part)
KT = H // 128            # 4 k-tiles
MT = R // 128            # 25 m-tiles
CH = 5                   # m-tiles per lse chunk
NCH = MT // CH           # 5 chunks
NW = 500                 # main n-block width (<=512 f32 per psum bank)
NQ = 4                   # psum pairs per m-tile (2 n-blocks each)
EXT_N = 500
EXT_NB = VE // EXT_N     # 10

LOG_LO = float(np.log(0.001))
LOG_HI = float(np.log(1.0 - 0.001))

_CACHE = {}


def _dedupe_act_table_loads(nc):
    """Collapse activation-table thrash: point every load at a table that
    serves its following activations when one exists, then drop loads that
    re-load the already-loaded table. Saves ~1.8us per removed load on ACT."""
    from concourse.hw_specs import get_activation_tables
    tables = list(get_activation_tables(nc.m.arch).items())
    for blk in nc.m.functions[0].blocks:
        insts = blk.instructions
        loads = [(i, inst) for i, inst in enumerate(insts)
                 if isinstance(inst, mybir.InstLoadActFuncSet)]
        if not loads:
            continue
        for li, (pos, inst) in enumerate(loads):
            end = loads[li + 1][0] if li + 1 < len(loads) else len(insts)
            funcs = {s.func for s in insts[pos:end]
                     if isinstance(s, mybir.InstActivation)}
            if not funcs:
                continue
            want = funcs | {AF.Exp, AF.Ln, AF.Identity, AF.Copy}
            pick = None
            for tid, (name, fs) in enumerate(tables):
                if want <= fs:
                    pick = tid
                    break
            if pick is None:
                for tid, (name, fs) in enumerate(tables):
                    if funcs <= fs:
                        pick = tid
                        break
            if pick is not None:
                inst.act_func_set_id = pick
        cur = None
        to_drop = []
        for pos, inst in loads:
            if cur is not None and inst.act_func_set_id == cur:
                si = inst.sync_info
                clean = si is None or (not si.on_wait and not si.on_update)
                if clean:
                    to_drop.append(inst)
                    continue
            cur = inst.act_func_set_id
        for inst in to_drop:
            insts.remove(inst)


def _build(with_bias):
    nc = bacc.Bacc("TRN2", target_bir_lowering=False, debug=False,
                   num_devices=N_CORES)

    hT = [nc.dram_tensor(f"hT{k}", [128, R], BF16, kind="ExternalInput").ap()
          for k in range(KT)]
    wT = [nc.dram_tensor(f"wT{k}", [128, VL], BF16, kind="ExternalInput").ap()
          for k in range(KT)]
    if with_bias:
        brow = nc.dram_tensor("brow", [1, VL], BF16, kind="ExternalInput").ap()
    dT = nc.dram_tensor("dT", [KT, 128, RL], F32, kind="ExternalInput").ap()
    wcT = nc.dram_tensor("wcT", [KT, 128, 1], F32, kind="ExternalInput").ap()
    bc_t = nc.dram_tensor("bc", [1, 1], F32, kind="ExternalInput").ap()
    # attn hi/lo bf16 split: [2, S, BL*T]
    attnT = nc.dram_tensor("attnT", [2, S, BL * T], BF16, kind="ExternalInput").ap()
    idx_t = nc.dram_tensor("idx", [S, BL], F32, kind="ExternalInput").ap()

    out_tgt = nc.dram_tensor("out_tgt", [R, VL], F32, kind="ExternalOutput").ap()
    out_ext = nc.dram_tensor("out_ext", [BL, T, VE], F32, kind="ExternalOutput").ap()

    cc_in = [nc.dram_tensor(f"cc_in{g}", [128, CH], F32).ap() for g in range(NCH)]
    cc_out = [nc.dram_tensor(f"cc_out{g}", [128, CH], F32, addr_space="Shared").ap()
              for g in range(NCH)]

    core_ids = list(range(N_CORES))

    with tile.TileContext(nc) as tc, ExitStack() as ctx:
        const = ctx.enter_context(tc.tile_pool(name="const", bufs=1))
        xpool = ctx.enter_context(tc.tile_pool(name="x", bufs=8))
        epool = ctx.enter_context(tc.tile_pool(name="E", bufs=1))
        outpool = ctx.enter_context(tc.tile_pool(name="out", bufs=3))
        statpool = ctx.enter_context(tc.tile_pool(name="stat", bufs=2))
        ohpool = ctx.enter_context(tc.tile_pool(name="oh", bufs=2))
        extstage = ctx.enter_context(tc.tile_pool(name="exts", bufs=4))
        ps_main = ctx.enter_context(tc.tile_pool(name="psm", bufs=3, space="PSUM"))
        ps_ext = ctx.enter_context(tc.tile_pool(name="pse", bufs=2, space="PSUM"))

        # ---- persistent SBUF loads (per-k tiles so PE can start early) ----
        hT_sb = [const.tile([128, R], BF16, name=f"hts{k}") for k in range(KT)]
        wT_sb = [const.tile([128, VL], BF16, name=f"wts{k}") for k in range(KT)]
        for k in range(KT):
            nc.sync.dma_start(wT_sb[k][:], wT[k])
            nc.sync.dma_start(hT_sb[k][:], hT[k])
        if with_bias:
            b_sb = const.tile([1, VL], BF16)
            nc.sync.dma_start(b_sb[:], brow[:])
            ones_sb = const.tile([1, 128], BF16)
            nc.vector.memset(ones_sb[:], 1.0)
        dT_sb = const.tile([128, KT * RL], F32)
        for k in range(KT):
            nc.sync.dma_start(dT_sb[:, k * RL:(k + 1) * RL], dT[k])
        wcT_sb = const.tile([128, KT], F32)
        for k in range(KT):
            nc.sync.dma_start(wcT_sb[:, k:k + 1], wcT[k])
        bc_sb = const.tile([1, 1], F32)
        nc.sync.dma_start(bc_sb[:], bc_t[:])
        ones50 = const.tile([1, 64], F32)
        nc.vector.memset(ones50[:], 1.0)
        attnT_sb = const.tile([S, 2 * BL * T], BF16)
        nc.sync.dma_start(attnT_sb[:, :BL * T], attnT[0])
        nc.sync.dma_start(attnT_sb[:, BL * T:], attnT[1])
        idx_sb = const.tile([S, BL], F32)
        nc.sync.dma_start(idx_sb[:], idx_t[:])
        iota_sb = const.tile([S, VE], F32)
        nc.gpsimd.iota(iota_sb[:], pattern=[[1, VE]], base=0, channel_multiplier=0,
                       allow_small_or_imprecise_dtypes=True)
        # kill ext-vocab column 0 (UNK): make it unmatchable
        nc.gpsimd.memset(iota_sb[:, 0:1], -1.0)

        # ---- copy gate: g1m[t, b] = 1 - sigmoid(dec[t,b] . Wc + bc) ----
        g1m = const.tile([64, BL], F32)
        for b in range(BL):
            gp = ps_ext.tile([64, EXT_N], F32, tag="eps")
            for k in range(KT):
                lhs = dT_sb[:, k * RL + b: k * RL + b + (T - 1) * BL + 1: BL]
                nc.tensor.matmul(gp[:T, 0:1], lhsT=lhs, rhs=wcT_sb[:, k:k + 1],
                                 start=(k == 0), stop=False)
            nc.tensor.matmul(gp[:T, 0:1], lhsT=ones50[:, :T], rhs=bc_sb[:],
                             start=False, stop=True)
            sig = extstage.tile([64, EXT_N], F32, tag="exts")
            nc.scalar.activation(sig[:T, 0:1], gp[:T, 0:1], AF.Sigmoid)
            nc.vector.tensor_scalar(g1m[:T, b:b + 1], sig[:T, 0:1], -1.0, 1.0,
                                    OP.mult, OP.add)

        # ---- ext part emitter (per local batch element) ----
        def emit_ext(b):
            oh = ohpool.tile([S, VE], BF16)
            nc.gpsimd.tensor_scalar(oh[:], iota_sb[:], idx_sb[:, b:b + 1], None,
                                    OP.is_equal)
            for nb in range(EXT_NB):
                ps = ps_ext.tile([64, EXT_N], F32, tag="eps")
                rhs = oh[:, nb * EXT_N:(nb + 1) * EXT_N]
                nc.tensor.matmul(ps[:T, :], lhsT=attnT_sb[:, b * T:(b + 1) * T],
                                 rhs=rhs, start=True, stop=False)
                nc.tensor.matmul(
                    ps[:T, :],
                    lhsT=attnT_sb[:, BL * T + b * T: BL * T + (b + 1) * T],
                    rhs=rhs, start=False, stop=True)
                st = extstage.tile([64, EXT_N], F32, tag="exts")
                # ext = raw * g1m; gate folded into Ln's per-partition scale,
                # clip done in log space (Ln(0) = -inf clips to LOG_LO)
                nc.scalar.activation(st[:T, :], ps[:T, :], AF.Ln,
                                     scale=g1m[:T, b:b + 1])
                nc.vector.tensor_scalar(st[:T, :], st[:T, :], LOG_LO, LOG_HI,
                                        OP.max, OP.min)
                nc.sync.dma_start(out_ext[b, :, nb * EXT_N:(nb + 1) * EXT_N],
                                  st[:T, :])

        # interleave ext batches between main chunks
        ext_sched = {0: [0, 1], 1: [2, 3], 2: [4, 5], 3: [6], 4: [7]}

        # ---- main: logits, online logsumexp, output ----
        for g in range(NCH):
            sums_g = statpool.tile([128, CH], F32, tag="sums")
            x_tiles = []
            for j in range(CH):
                m = g * CH + j
                x_m = xpool.tile([128, VL], BF16, tag="x")
                x_tiles.append(x_m)
                for q in range(NQ):
                    ps = ps_main.tile([128, 1024], F32)
                    for k in range(KT):
                        for nn in range(2):
                            n = 2 * q + nn
                            last = (k == KT - 1) and not with_bias
                            nc.tensor.matmul(
                                ps[:, nn * 512: nn * 512 + NW],
                                lhsT=hT_sb[k][:, m * 128:(m + 1) * 128],
                                rhs=wT_sb[k][:, n * NW:(n + 1) * NW],
                                start=(k == 0), stop=last)
                    if with_bias:
                        for nn in range(2):
                            n = 2 * q + nn
                            nc.tensor.matmul(
                                ps[:, nn * 512: nn * 512 + NW],
                                lhsT=ones_sb[:],
                                rhs=b_sb[:, n * NW:(n + 1) * NW],
                                start=False, stop=True)
                    # psum pair -> x (bf16), one strided copy per pair
                    src = ps[:].rearrange("p (b n) -> p b n", b=2)[:, :, :NW]
                    dst = x_m[:, q * 2 * NW:(q + 1) * 2 * NW].rearrange(
                        "p (b n) -> p b n", b=2)
                    if q % 2 == 0:
                        nc.vector.tensor_copy(dst, src)
                    else:
                        nc.scalar.copy(dst, src)
                E = epool.tile([128, VL], BF16, tag="E")
                nc.scalar.activation(E[:], x_m[:], AF.Exp,
                                     accum_out=sums_g[:, j:j + 1])

            nc.sync.dma_start(cc_in[g][:], sums_g[:])
            nc.gpsimd.collective_compute(
                "AllReduce", OP.add,
                replica_groups=[core_ids],
                ins=[cc_in[g][:]], outs=[cc_out[g][:]])
            tot_g = statpool.tile([128, CH], F32, tag="tot")
            nc.sync.dma_start(tot_g[:], cc_out[g][:])
            neglse = statpool.tile([128, CH], F32, tag="lse")
            nc.scalar.activation(neglse[:], tot_g[:], AF.Ln)
            nc.vector.tensor_scalar(neglse[:], neglse[:], -1.0, None, OP.mult)

            for j in range(CH):
                m = g * CH + j
                x_m = x_tiles[j]
                for half in range(2):
                    o = outpool.tile([128, VL // 2], F32)
                    src = x_m[:, half * (VL // 2):(half + 1) * (VL // 2)]
                    if half == 0:
                        nc.scalar.activation(o[:], src, AF.Identity,
                                             bias=neglse[:, j:j + 1])
                    else:
                        nc.vector.tensor_scalar(o[:], src, neglse[:, j:j + 1],
                                                None, OP.add)
                    nc.sync.dma_start(
                        out_tgt[m * 128:(m + 1) * 128,
                                half * (VL // 2):(half + 1) * (VL // 2)],
                        o[:])

            for b in ext_sched.get(g, []):
                emit_ext(b)

    nc.compile()
    _dedupe_act_table_loads(nc)
    return nc


def _get_nc(with_bias=False):
    key = ("nc", with_bias)
    if key not in _CACHE:
        _CACHE[key] = _build(with_bias)
    return _CACHE[key]


def kernel(**inputs):
    hidden = np.asarray(inputs["hidden"], dtype=np.float32)
    dec = np.asarray(inputs["dec_rnn_output"], dtype=np.float32)
    attn = np.asarray(inputs["attn"], dtype=np.float32)
    c2e = np.asarray(inputs["copy_to_ext"])
    W = np.asarray(inputs["W"], dtype=np.float32)
    bvec = np.asarray(inputs["b"], dtype=np.float32)
    Wc = np.asarray(inputs["Wc"], dtype=np.float32)
    bc = np.asarray(inputs["bc"], dtype=np.float32)

    with_bias = bool(np.any(bvec))
    bf = ml_dtypes.bfloat16
    hT_np = np.ascontiguousarray(
        hidden.reshape(R, H).T.reshape(KT, 128, R)).astype(bf)
    wcT_np = np.ascontiguousarray(Wc.reshape(1, H).T.reshape(KT, 128, 1))
    bc_np = bc.reshape(1, 1)

    in_maps = []
    for c in range(N_CORES):
        vs = slice(c * VL, (c + 1) * VL)
        bs = slice(c * BL, (c + 1) * BL)
        wT_np = np.ascontiguousarray(W[vs].T.reshape(KT, 128, VL)).astype(bf)
        dT_np = np.ascontiguousarray(
            dec[:, bs, :].reshape(RL, H).T.reshape(KT, 128, RL))
        # attnT[s, b*T + t] = attn[t, c*BL+b, s]; hi/lo bf16 split
        at = np.ascontiguousarray(
            attn[:, bs, :].transpose(2, 1, 0).reshape(S, BL * T))
        at_hi = at.astype(bf)
        at_lo = (at - at_hi.astype(np.float32)).astype(bf)
        attnT_np = np.ascontiguousarray(np.stack([at_hi, at_lo]))
        idx_np = np.ascontiguousarray(c2e[:, bs]).astype(np.float32)
        m = {"dT": dT_np, "wcT": wcT_np, "bc": bc_np,
             "attnT": attnT_np, "idx": idx_np}
        for k in range(KT):
            m[f"hT{k}"] = np.ascontiguousarray(hT_np[k])
            m[f"wT{k}"] = np.ascontiguousarray(wT_np[k])
        if with_bias:
            m["brow"] = bvec[vs].reshape(1, VL).astype(bf)
        in_maps.append(m)

    nc = _get_nc(with_bias)
    res = run_bass_kernel_spmd(nc, in_maps, core_ids=list(range(N_CORES)))

    out = np.empty((T, B, VT + VE), dtype=np.float32)
    for c in range(N_CORES):
        r = res.results[c]
        out[:, :, c * VL:(c + 1) * VL] = r["out_tgt"].reshape(T, B, VL)
        out[:, c * BL:(c + 1) * BL, VT:] = r["out_ext"].transpose(1, 0, 2)
    return out



# revision 4
# speedup vs baseline: 1.0434x; 1.0434x over previous
"""Trainium2 Bass kernel for nn_CopyGenerator (scatter_memory).

Computation (see the reference):
  out_tgt = log_softmax(hidden @ W.T + b)                    [T,B,VT]
  gate1m  = 1 - sigmoid(dec @ Wc.T + bc)                     [T,B]
  ext[t,b,v] = gate1m[t,b] * sum_s attn[t,b,s]*(idx[s,b]==v), idx==UNK masked
  out_ext = log(clip(ext, 1e-3, 1-1e-3))                     [T,B,VE]
  out = concat([out_tgt, out_ext], -1)

Sharding (8 cores):
  - Big matmul + log_softmax: column-parallel over tgt vocab (each core owns a
    4000-wide W slice, SBUF-resident bf16; all 3200 rows). The softmax
    denominator needs the full-vocab sum -> per-chunk partial row sums are
    AllReduced across cores (5 tiny pipelined collectives).
  - Copy-gate + scatter-add over ext vocab: data-parallel over batch (8 batch
    elements per core). The scatter-add is aw.T @ onehot(idx) on the tensor
    engine (exact, handles duplicate indices); attn is fed as a bf16 hi/lo
    pair so the ext sums are fp32-accurate. The whole ext phase runs as a
    tail that overlaps the last chunk's AllReduce.

Performance notes (vs the previous version):
  - outputs are written bf16 and widened to f32 on the host (halves out DMA).
  - psum drains are 2 big strided DVE copies per tile; the final
    out = x - ln(tot) runs in-place on DVE in the packed-bf16 fast mode.
  - chunk g's post-collective work is emitted during chunk g+1's compute so
    the ~28us collective latency is hidden by the pipeline.
  - DMA issues are spread across sequencers: loads on SP, collective staging
    on ACT/SP, output stores on Pool.
"""

import sys

if "/opt/trn_rl_repo" not in sys.path:
    sys.path.insert(0, "/opt/trn_rl_repo")

from contextlib import ExitStack

import ml_dtypes
import numpy as np

import concourse.bass as bass
import concourse.mybir as mybir
import concourse.tile as tile
from concourse import bacc
from concourse.bass_utils import run_bass_kernel_spmd

F32 = mybir.dt.float32
BF16 = mybir.dt.bfloat16
I16 = mybir.dt.int16
AF = mybir.ActivationFunctionType
OP = mybir.AluOpType

T, B, S, H = 50, 64, 100, 512
VT, VE = 32000, 5000
N_CORES = 8
VL = VT // N_CORES       # 4000 vocab cols per core
BL = B // N_CORES        # 8 batch per core (ext part)
R = T * B                # 3200 rows
RL = T * BL              # 400 rows (ext part)
KT = H // 128            # 4 k-tiles
MT = R // 128            # 25 m-tiles
CH = 5                   # m-tiles per lse chunk
NCH = MT // CH           # 5 chunks
NW = 500                 # matmul n-block width (<=512 f32 per psum bank)
NB = VL // NW            # 8 n-blocks per tile

LOG_LO = float(np.log(0.001))
LOG_HI = float(np.log(1.0 - 0.001))

_CACHE = {}


def _dedupe_act_table_loads(nc):
    """Collapse activation-table thrash: point every load at a table that
    serves its following activations when one exists, then drop loads that
    re-load the already-loaded table. Saves ~1.8us per removed load on ACT."""
    from concourse.hw_specs import get_activation_tables
    tables = list(get_activation_tables(nc.m.arch).items())
    for blk in nc.m.functions[0].blocks:
        insts = blk.instructions
        loads = [(i, inst) for i, inst in enumerate(insts)
                 if isinstance(inst, mybir.InstLoadActFuncSet)]
        if not loads:
            continue
        for li, (pos, inst) in enumerate(loads):
            end = loads[li + 1][0] if li + 1 < len(loads) else len(insts)
            funcs = {s.func for s in insts[pos:end]
                     if isinstance(s, mybir.InstActivation)}
            if not funcs:
                continue
            want = funcs | {AF.Exp, AF.Ln, AF.Identity, AF.Copy}
            pick = None
            for tid, (name, fs) in enumerate(tables):
                if want <= fs:
                    pick = tid
                    break
            if pick is None:
                for tid, (name, fs) in enumerate(tables):
                    if funcs <= fs:
                        pick = tid
                        break
            if pick is not None:
                inst.act_func_set_id = pick
        cur = None
        to_drop = []
        for pos, inst in loads:
            if cur is not None and inst.act_func_set_id == cur:
                si = inst.sync_info
                clean = si is None or (not si.on_wait and not si.on_update)
                if clean:
                    to_drop.append(inst)
                    continue
            cur = inst.act_func_set_id
        for inst in to_drop:
            insts.remove(inst)


def _build(with_bias):
    nc = bacc.Bacc("TRN2", target_bir_lowering=False, debug=False,
                   num_devices=N_CORES)

    hT = [nc.dram_tensor(f"hT{k}", [128, R], BF16, kind="ExternalInput").ap()
          for k in range(KT)]
    wT = [nc.dram_tensor(f"wT{k}", [128, VL], BF16, kind="ExternalInput").ap()
          for k in range(KT)]
    if with_bias:
        brow = nc.dram_tensor("brow", [1, VL], BF16, kind="ExternalInput").ap()
    dT = nc.dram_tensor("dT", [KT, 128, RL], F32, kind="ExternalInput").ap()
    wcT = nc.dram_tensor("wcT", [KT, 128, 1], F32, kind="ExternalInput").ap()
    bc_t = nc.dram_tensor("bc", [1, 1], F32, kind="ExternalInput").ap()
    # attn hi/lo bf16 split: [2, S, BL*T] (b-major cols)
    attnT = nc.dram_tensor("attnT", [2, S, BL * T], BF16, kind="ExternalInput").ap()
    idx_t = nc.dram_tensor("idx", [S, BL], F32, kind="ExternalInput").ap()

    out_tgt = nc.dram_tensor("out_tgt", [R, VL], BF16, kind="ExternalOutput").ap()
    out_ext = nc.dram_tensor("out_ext", [BL, T, VE], BF16, kind="ExternalOutput").ap()

    cc_in = [nc.dram_tensor(f"cc_in{g}", [128, CH], F32).ap() for g in range(NCH)]
    cc_out = [nc.dram_tensor(f"cc_out{g}", [128, CH], F32, addr_space="Shared").ap()
              for g in range(NCH)]

    core_ids = list(range(N_CORES))

    with tile.TileContext(nc) as tc, ExitStack() as ctx:
        const = ctx.enter_context(tc.tile_pool(name="const", bufs=1))
        xpool = ctx.enter_context(tc.tile_pool(name="x", bufs=11))
        epool = ctx.enter_context(tc.tile_pool(name="E", bufs=1))
        statpool = ctx.enter_context(tc.tile_pool(name="stat", bufs=2))
        lnpool = ctx.enter_context(tc.tile_pool(name="lng", bufs=2))
        ohpool = ctx.enter_context(tc.tile_pool(name="oh", bufs=2))
        extstage = ctx.enter_context(tc.tile_pool(name="exts", bufs=2))
        pspool = ctx.enter_context(tc.tile_pool(name="ps", bufs=2, space="PSUM"))

        # ---- persistent SBUF loads (per-k tiles so PE can start early) ----
        hT_sb = [const.tile([128, R], BF16, name=f"hts{k}") for k in range(KT)]
        wT_sb = [const.tile([128, VL], BF16, name=f"wts{k}") for k in range(KT)]
        for k in range(KT):
            nc.sync.dma_start(wT_sb[k][:], wT[k])
            nc.sync.dma_start(hT_sb[k][:], hT[k])
        if with_bias:
            b_sb = const.tile([1, VL], BF16)
            nc.sync.dma_start(b_sb[:], brow[:])
            ones_sb = const.tile([1, 128], BF16)
            nc.vector.memset(ones_sb[:], 1.0)
        dT_sb = const.tile([128, KT * RL], F32)
        for k in range(KT):
            nc.sync.dma_start(dT_sb[:, k * RL:(k + 1) * RL], dT[k])
        wcT_sb = const.tile([128, KT], F32)
        for k in range(KT):
            nc.sync.dma_start(wcT_sb[:, k:k + 1], wcT[k])
        bc_sb = const.tile([1, 1], F32)
        nc.sync.dma_start(bc_sb[:], bc_t[:])
        onesT = const.tile([1, 64], F32)
        nc.vector.memset(onesT[:], 1.0)
        attnT_sb = const.tile([S, 2 * BL * T], BF16)
        nc.sync.dma_start(attnT_sb[:, :BL * T], attnT[0])
        nc.sync.dma_start(attnT_sb[:, BL * T:], attnT[1])
        idx_sb = const.tile([S, BL], F32)
        nc.sync.dma_start(idx_sb[:], idx_t[:])
        iota_sb = const.tile([S, VE], I16)
        nc.gpsimd.iota(iota_sb[:], pattern=[[1, VE]], base=0, channel_multiplier=0,
                       allow_small_or_imprecise_dtypes=True)
        # kill ext-vocab column 0 (UNK): make it unmatchable
        nc.gpsimd.memset(iota_sb[:, 0:1], -1.0)

        # ---- copy gate: g1m[t, b] = 1 - sigmoid(dec[t,b] . Wc + bc) ----
        # dT is b-major ([.., b*T + t]) so each b's 50 rows are contiguous.
        gp = pspool.tile([128, 2048], F32, tag="ps")
        for b in range(BL):
            for k in range(KT):
                lhs = dT_sb[:, k * RL + b * T: k * RL + (b + 1) * T]
                nc.tensor.matmul(gp[:T, b:b + 1], lhsT=lhs,
                                 rhs=wcT_sb[:, k:k + 1],
                                 start=(k == 0), stop=False)
            nc.tensor.matmul(gp[:T, b:b + 1], lhsT=onesT[:, :T], rhs=bc_sb[:],
                             start=False, stop=True)
        sig = const.tile([T, BL], F32)
        nc.scalar.activation(sig[:], gp[:T, :BL], AF.Sigmoid)
        g1m = const.tile([T, BL], F32)
        nc.vector.tensor_scalar(g1m[:], sig[:], -1.0, 1.0, OP.mult, OP.add)

        # ---- main: logits, online logsumexp, pipelined output ----
        sums = [None] * NCH
        tots = [None] * NCH
        lngs = [None] * NCH
        xs = [[None] * CH for _ in range(NCH)]

        def emit_chunk(g):
            sums_g = statpool.tile([128, CH], F32, tag="sums")
            sums[g] = sums_g
            for j in range(CH):
                m = g * CH + j
                x_m = xpool.tile([128, VL], BF16, tag="x")
                xs[g][j] = x_m
                for half in range(2):
                    ps = pspool.tile([128, 2048], F32, tag="ps")
                    for k in range(KT):
                        for q in range(4):
                            n = half * 4 + q
                            nc.tensor.matmul(
                                ps[:, q * 512: q * 512 + NW],
                                lhsT=hT_sb[k][:, m * 128:(m + 1) * 128],
                                rhs=wT_sb[k][:, n * NW:(n + 1) * NW],
                                start=(k == 0),
                                stop=(k == KT - 1) and not with_bias)
                    if with_bias:
                        for q in range(4):
                            n = half * 4 + q
                            nc.tensor.matmul(
                                ps[:, q * 512: q * 512 + NW],
                                lhsT=ones_sb[:],
                                rhs=b_sb[:, n * NW:(n + 1) * NW],
                                start=False, stop=True)
                    src = ps[:].rearrange("p (b n) -> p b n", b=4)[:, :, :NW]
                    dst = x_m[:, half * 4 * NW:(half + 1) * 4 * NW].rearrange(
                        "p (b n) -> p b n", b=4)
                    nc.vector.tensor_copy(dst, src)
                E = epool.tile([128, VL], BF16, tag="E")
                nc.scalar.activation(E[:], x_m[:], AF.Exp,
                                     accum_out=sums_g[:, j:j + 1])
            # collective staging: store partial sums (ACT just produced them),
            # all-reduce on pool, load the total back on SP.
            nc.scalar.dma_start(cc_in[g][:], sums_g[:])
            nc.gpsimd.collective_compute(
                "AllReduce", OP.add,
                replica_groups=[core_ids],
                ins=[cc_in[g][:]], outs=[cc_out[g][:]])
            tot_g = statpool.tile([128, CH], F32, tag="tot")
            tots[g] = tot_g
            nc.sync.dma_start(tot_g[:], cc_out[g][:])

        def emit_output(g):
            # ln of the all-reduced sums; then out = x - ln(tot) in-place on
            # DVE (packed-bf16 fast mode) and store from the Pool queue.
            lng = lnpool.tile([128, CH], F32, tag="lng")
            lngs[g] = lng
            nc.scalar.activation(lng[:], tots[g][:], AF.Ln)
            for j in range(CH):
                m = g * CH + j
                x_m = xs[g][j]
                nc.vector.tensor_scalar(x_m[:], x_m[:], lng[:, j:j + 1],
                                        None, OP.subtract)
                nc.gpsimd.dma_start(out_tgt[m * 128:(m + 1) * 128, :], x_m[:])

        for g in range(NCH):
            emit_chunk(g)
            if g >= 1:
                emit_output(g - 1)

        # ---- ext tail: scatter-add via one-hot matmul, overlaps last cc ----
        # per b: one-hot on DVE (int16 iota, 4x mode), 10 n-blocks of 500 in
        # groups of 4/4/2 per psum tile; Ln(scale=g1m) reads psum directly,
        # clamp in-place on DVE, one DMA per b from Pool.
        for b in range(BL):
            oh = ohpool.tile([S, VE], BF16, tag="oh")
            nc.vector.tensor_scalar(oh[:], iota_sb[:], idx_sb[:, b:b + 1],
                                    None, OP.is_equal)
            st = extstage.tile([T, VE], BF16, tag="st")
            for grp, gw in ((0, 4), (4, 4), (8, 2)):
                ps = pspool.tile([128, 2048], F32, tag="ps")
                for hl in range(2):
                    lhsT = attnT_sb[:, hl * BL * T + b * T:
                                    hl * BL * T + (b + 1) * T]
                    for q in range(gw):
                        nb = grp + q
                        nc.tensor.matmul(
                            ps[:T, q * 512: q * 512 + NW],
                            lhsT=lhsT,
                            rhs=oh[:, nb * NW:(nb + 1) * NW],
                            start=(hl == 0), stop=(hl == 1))
                src = ps[:T].rearrange("p (b n) -> p b n", b=4)[:, :gw, :NW]
                dst = st[:, grp * NW:(grp + gw) * NW].rearrange(
                    "p (b n) -> p b n", b=gw)
                # ext = raw * g1m folded into Ln's per-partition scale;
                # clip is done in log space (Ln(0) = -inf clips to LOG_LO)
                nc.scalar.activation(dst, src, AF.Ln, scale=g1m[:, b:b + 1])
            nc.vector.tensor_scalar(st[:], st[:], LOG_LO, LOG_HI,
                                    OP.max, OP.min)
            nc.gpsimd.dma_start(out_ext[b], st[:])

        # last chunk's output after the ext work so ACT/DVE stay busy while
        # the final collective is in flight.
        emit_output(NCH - 1)

    nc.compile()
    _dedupe_act_table_loads(nc)
    return nc


def _get_nc(with_bias=False):
    key = ("nc", with_bias)
    if key not in _CACHE:
        _CACHE[key] = _build(with_bias)
    return _CACHE[key]


def kernel(**inputs):
    hidden = np.asarray(inputs["hidden"], dtype=np.float32)
    dec = np.asarray(inputs["dec_rnn_output"], dtype=np.float32)
    attn = np.asarray(inputs["attn"], dtype=np.float32)
    c2e = np.asarray(inputs["copy_to_ext"])
    W = np.asarray(inputs["W"], dtype=np.float32)
    bvec = np.asarray(inputs["b"], dtype=np.float32)
    Wc = np.asarray(inputs["Wc"], dtype=np.float32)
    bc = np.asarray(inputs["bc"], dtype=np.float32)

    with_bias = bool(np.any(bvec))
    bf = ml_dtypes.bfloat16
    hT_np = np.ascontiguousarray(
        hidden.reshape(R, H).T.reshape(KT, 128, R)).astype(bf)
    wcT_np = np.ascontiguousarray(Wc.reshape(1, H).T.reshape(KT, 128, 1))
    bc_np = bc.reshape(1, 1)

    in_maps = []
    for c in range(N_CORES):
        vs = slice(c * VL, (c + 1) * VL)
        bs = slice(c * BL, (c + 1) * BL)
        wT_np = np.ascontiguousarray(W[vs].T.reshape(KT, 128, VL)).astype(bf)
        # dT rows are b-major: r = b*T + t
        dT_np = np.ascontiguousarray(
            dec[:, bs, :].transpose(1, 0, 2).reshape(RL, H).T.reshape(
                KT, 128, RL))
        # attnT[s, b*T + t] = attn[t, c*BL+b, s]; hi/lo bf16 split
        at = np.ascontiguousarray(
            attn[:, bs, :].transpose(2, 1, 0).reshape(S, BL * T))
        at_hi = at.astype(bf)
        at_lo = (at - at_hi.astype(np.float32)).astype(bf)
        attnT_np = np.ascontiguousarray(np.stack([at_hi, at_lo]))
        idx_np = np.ascontiguousarray(c2e[:, bs]).astype(np.float32)
        m = {"dT": dT_np, "wcT": wcT_np, "bc": bc_np,
             "attnT": attnT_np, "idx": idx_np}
        for k in range(KT):
            m[f"hT{k}"] = np.ascontiguousarray(hT_np[k])
            m[f"wT{k}"] = np.ascontiguousarray(wT_np[k])
        if with_bias:
            m["brow"] = bvec[vs].reshape(1, VL).astype(bf)
        in_maps.append(m)

    nc = _get_nc(with_bias)
    res = run_bass_kernel_spmd(nc, in_maps, core_ids=list(range(N_CORES)))

    out = np.empty((T, B, VT + VE), dtype=np.float32)
    for c in range(N_CORES):
        r = res.results[c]
        out[:, :, c * VL:(c + 1) * VL] = np.asarray(
            r["out_tgt"], dtype=np.float32).reshape(T, B, VL)
        out[:, c * BL:(c + 1) * BL, VT:] = np.asarray(
            r["out_ext"], dtype=np.float32).transpose(1, 0, 2)
    return out


# revision 6
# speedup vs baseline: 1.3341x; 1.2786x over previous
"""Trainium2 Bass kernel for nn_CopyGenerator (scatter_memory).

Computation (see the reference):
  out_tgt = log_softmax(hidden @ W.T + b)                    [T,B,VT]
  gate1m  = 1 - sigmoid(dec @ Wc.T + bc)                     [T,B]
  ext[t,b,v] = gate1m[t,b] * sum_s attn[t,b,s]*(idx[s,b]==v), idx==UNK masked
  out_ext = log(clip(ext, 1e-3, 1-1e-3))                     [T,B,VE]
  out = concat([out_tgt, out_ext], -1)

Sharding (8 cores):
  - Big matmul + log_softmax: column-parallel over tgt vocab (each core owns a
    4000-wide W slice, SBUF-resident bf16; all 3200 rows). The softmax
    denominator needs the full-vocab sum -> per-chunk partial row sums are
    AllGathered across cores (cheaper than AllReduce in latency) and summed
    locally; 5 pipelined collectives.
  - Copy-gate + scatter-add over ext vocab: data-parallel over batch (8 batch
    elements per core). The scatter-add is aw.T @ onehot(idx) on the tensor
    engine (exact, handles duplicate indices); attn is fed as a bf16 hi/lo
    pair so the ext sums are fp32-accurate. Batch elements are processed in
    pairs packed into 64-row psum blocks (rows 50..63 are zero padding) so
    the Ln/clamp passes cover two batch elements per op. The ext phase runs
    as a tail that overlaps the last chunk's collective.

Performance structure:
  - outputs are written bf16 and widened to f32 on the host (halves out DMA).
  - psum drains are 2 big strided DVE copies per tile; the final
    out = x - ln(tot) runs in-place on DVE in the packed-bf16 fast mode.
  - chunk g's post-collective work is emitted during chunk g+1's compute so
    the collective latency is hidden by the pipeline.
  - DMA issues are spread across sequencers: loads on SP, collective staging
    on ACT/SP, output stores on Pool.
"""

import sys

if "/opt/trn_rl_repo" not in sys.path:
    sys.path.insert(0, "/opt/trn_rl_repo")

from contextlib import ExitStack

import ml_dtypes
import numpy as np

import concourse.bass as bass
import concourse.mybir as mybir
import concourse.tile as tile
from concourse import bacc
from concourse.bass_utils import run_bass_kernel_spmd

F32 = mybir.dt.float32
BF16 = mybir.dt.bfloat16
I16 = mybir.dt.int16
AF = mybir.ActivationFunctionType
OP = mybir.AluOpType

T, B, S, H = 50, 64, 100, 512
VT, VE = 32000, 5000
N_CORES = 8
VL = VT // N_CORES       # 4000 vocab cols per core
BL = B // N_CORES        # 8 batch per core (ext part)
R = T * B                # 3200 rows
BP = 64                  # padded rows per batch element (T=50 -> 64)
RL = BL * BP             # 512 padded gate/attn cols per core
KT = H // 128            # 4 k-tiles
MT = R // 128            # 25 m-tiles
CH = 5                   # m-tiles per lse chunk
NCH = MT // CH           # 5 chunks
NW = 500                 # matmul n-block width (<=512 f32 per psum bank)
NP = BL // 2             # 4 ext batch pairs

LOG_LO = float(np.log(0.001))
LOG_HI = float(np.log(1.0 - 0.001))

_CACHE = {}


def _dedupe_act_table_loads(nc):
    """Collapse activation-table thrash: point every load at a table that
    serves its following activations when one exists, then drop loads that
    re-load the already-loaded table. Saves ~1.8us per removed load on ACT."""
    from concourse.hw_specs import get_activation_tables
    tables = list(get_activation_tables(nc.m.arch).items())
    for blk in nc.m.functions[0].blocks:
        insts = blk.instructions
        loads = [(i, inst) for i, inst in enumerate(insts)
                 if isinstance(inst, mybir.InstLoadActFuncSet)]
        if not loads:
            continue
        for li, (pos, inst) in enumerate(loads):
            end = loads[li + 1][0] if li + 1 < len(loads) else len(insts)
            funcs = {s.func for s in insts[pos:end]
                     if isinstance(s, mybir.InstActivation)}
            if not funcs:
                continue
            want = funcs | {AF.Exp, AF.Ln, AF.Identity, AF.Copy}
            pick = None
            for tid, (name, fs) in enumerate(tables):
                if want <= fs:
                    pick = tid
                    break
            if pick is None:
                for tid, (name, fs) in enumerate(tables):
                    if funcs <= fs:
                        pick = tid
                        break
            if pick is not None:
                inst.act_func_set_id = pick
        cur = None
        to_drop = []
        for pos, inst in loads:
            if cur is not None and inst.act_func_set_id == cur:
                si = inst.sync_info
                clean = si is None or (not si.on_wait and not si.on_update)
                if clean:
                    to_drop.append(inst)
                    continue
            cur = inst.act_func_set_id
        for inst in to_drop:
            insts.remove(inst)


def _build(with_bias):
    nc = bacc.Bacc("TRN2", target_bir_lowering=False, debug=False,
                   num_devices=N_CORES)

    hT = [nc.dram_tensor(f"hT{k}", [128, R], BF16, kind="ExternalInput").ap()
          for k in range(KT)]
    wT = [nc.dram_tensor(f"wT{k}", [128, VL], BF16, kind="ExternalInput").ap()
          for k in range(KT)]
    if with_bias:
        brow = nc.dram_tensor("brow", [1, VL], BF16, kind="ExternalInput").ap()
    dT = nc.dram_tensor("dT", [KT, 128, RL], F32, kind="ExternalInput").ap()
    wcT = nc.dram_tensor("wcT", [KT, 128, 1], F32, kind="ExternalInput").ap()
    bc_t = nc.dram_tensor("bc", [1, 1], F32, kind="ExternalInput").ap()
    # attn hi/lo bf16 split, 64-padded b-major cols: [2, S, BL*64]
    attnT = nc.dram_tensor("attnT", [2, S, RL], BF16, kind="ExternalInput").ap()
    idx_t = nc.dram_tensor("idx", [S, BL], F32, kind="ExternalInput").ap()

    out_tgt = nc.dram_tensor("out_tgt", [R, VL], BF16, kind="ExternalOutput").ap()
    out_ext = nc.dram_tensor("out_ext", [BL, T, VE], BF16, kind="ExternalOutput").ap()

    cc_in = [nc.dram_tensor(f"cc_in{g}", [128, CH], F32).ap() for g in range(NCH)]
    cc_out = [nc.dram_tensor(f"cc_out{g}", [N_CORES, 128, CH], F32,
                             addr_space="Shared").ap()
              for g in range(NCH)]

    core_ids = list(range(N_CORES))

    with tile.TileContext(nc) as tc, ExitStack() as ctx:
        const = ctx.enter_context(tc.tile_pool(name="const", bufs=1))
        xpool = ctx.enter_context(tc.tile_pool(name="x", bufs=11))
        scr = ctx.enter_context(tc.tile_pool(name="scr", bufs=4))
        statpool = ctx.enter_context(tc.tile_pool(name="stat", bufs=2))
        gathpool = ctx.enter_context(tc.tile_pool(name="gath", bufs=2))
        lnpool = ctx.enter_context(tc.tile_pool(name="lng", bufs=2))
        pspool = ctx.enter_context(tc.tile_pool(name="ps", bufs=2, space="PSUM"))

        # ---- persistent SBUF loads; small gate/ext inputs first, then the
        # first matmul k-tiles, so PE can start as early as possible ----
        hT_sb = [const.tile([128, R], BF16, name=f"hts{k}") for k in range(KT)]
        wT_sb = [const.tile([128, VL], BF16, name=f"wts{k}") for k in range(KT)]
        dT_sb = const.tile([128, KT * RL], F32)
        for k in range(KT):
            nc.sync.dma_start(dT_sb[:, k * RL:(k + 1) * RL], dT[k])
        wcT_sb = const.tile([128, KT], F32)
        for k in range(KT):
            nc.sync.dma_start(wcT_sb[:, k:k + 1], wcT[k])
        bc_sb = const.tile([1, 1], F32)
        nc.sync.dma_start(bc_sb[:], bc_t[:])
        nc.sync.dma_start(wT_sb[0][:], wT[0])
        nc.sync.dma_start(hT_sb[0][:], hT[0])
        attnT_sb = const.tile([S, 2 * RL], BF16)
        nc.sync.dma_start(attnT_sb[:, :RL], attnT[0])
        nc.sync.dma_start(attnT_sb[:, RL:], attnT[1])
        idx_sb = const.tile([S, BL], F32)
        nc.sync.dma_start(idx_sb[:], idx_t[:])
        for k in range(1, KT):
            nc.sync.dma_start(wT_sb[k][:], wT[k])
            nc.sync.dma_start(hT_sb[k][:], hT[k])
        if with_bias:
            b_sb = const.tile([1, VL], BF16)
            nc.sync.dma_start(b_sb[:], brow[:])
            ones_sb = const.tile([1, 128], BF16)
            nc.vector.memset(ones_sb[:], 1.0)
        onesT = const.tile([1, 64], F32)
        nc.vector.memset(onesT[:], 1.0)
        iota_sb = const.tile([S, VE], I16)
        nc.gpsimd.iota(iota_sb[:], pattern=[[1, VE]], base=0, channel_multiplier=0,
                       allow_small_or_imprecise_dtypes=True)
        # kill ext-vocab column 0 (UNK): make it unmatchable
        nc.gpsimd.memset(iota_sb[:, 0:1], -1.0)

        # ---- copy gate: g1m[t, b] = 1 - sigmoid(dec[t,b] . Wc + bc) ----
        # dT cols are 64-padded b-major; pairs pack to psum partition halves
        # (b even -> rows 0..63, b odd -> rows 64..127), one column per pair.
        gp = pspool.tile([128, 2048], F32, tag="ps")
        for b in range(BL):
            p, half = b // 2, (b % 2) * BP
            for k in range(KT):
                lhs = dT_sb[:, k * RL + b * BP: k * RL + (b + 1) * BP]
                nc.tensor.matmul(gp[half:half + BP, p:p + 1], lhsT=lhs,
                                 rhs=wcT_sb[:, k:k + 1],
                                 start=(k == 0), stop=False)
            nc.tensor.matmul(gp[half:half + BP, p:p + 1], lhsT=onesT[:],
                             rhs=bc_sb[:], start=False, stop=True)
        sig = const.tile([128, NP], F32)
        nc.scalar.activation(sig[:], gp[:, :NP], AF.Sigmoid)
        g1m = const.tile([128, NP], F32)
        nc.vector.tensor_scalar(g1m[:], sig[:], -1.0, 1.0, OP.mult, OP.add)

        # ---- main: logits, online logsumexp, pipelined output ----
        tots = [None] * NCH
        gaths = [None] * NCH
        xs = [[None] * CH for _ in range(NCH)]

        def emit_chunk(g):
            sums_g = statpool.tile([128, CH], F32, tag="sums")
            for j in range(CH):
                m = g * CH + j
                x_m = xpool.tile([128, VL], BF16, tag="x")
                xs[g][j] = x_m
                for half in range(2):
                    ps = pspool.tile([128, 2048], F32, tag="ps")
                    for k in range(KT):
                        for q in range(4):
                            n = half * 4 + q
                            nc.tensor.matmul(
                                ps[:, q * 512: q * 512 + NW],
                                lhsT=hT_sb[k][:, m * 128:(m + 1) * 128],
                                rhs=wT_sb[k][:, n * NW:(n + 1) * NW],
                                start=(k == 0),
                                stop=(k == KT - 1) and not with_bias)
                    if with_bias:
                        for q in range(4):
                            n = half * 4 + q
                            nc.tensor.matmul(
                                ps[:, q * 512: q * 512 + NW],
                                lhsT=ones_sb[:],
                                rhs=b_sb[:, n * NW:(n + 1) * NW],
                                start=False, stop=True)
                    src = ps[:].rearrange("p (b n) -> p b n", b=4)[:, :, :NW]
                    dst = x_m[:, half * 4 * NW:(half + 1) * 4 * NW].rearrange(
                        "p (b n) -> p b n", b=4)
                    nc.vector.tensor_copy(dst, src)
                E = scr.tile([128, VE], BF16, tag="scr")
                nc.scalar.activation(E[:, :VL], x_m[:], AF.Exp,
                                     accum_out=sums_g[:, j:j + 1])
            # collective staging: store partial sums (ACT just produced them),
            # all-gather on pool, load the per-core blocks back on SP.
            nc.scalar.dma_start(cc_in[g][:], sums_g[:])
            nc.gpsimd.collective_compute(
                "AllGather", OP.bypass,
                replica_groups=[core_ids],
                ins=[cc_in[g][:]], outs=[cc_out[g][:]])
            gath_g = gathpool.tile([128, N_CORES * CH], F32, tag="gath")
            gaths[g] = gath_g
            for rr in range(N_CORES):
                nc.sync.dma_start(gath_g[:, rr * CH:(rr + 1) * CH],
                                  cc_out[g][rr])

        def emit_output(g):
            # local sum of the gathered per-core partials, ln, then
            # out = x - ln(tot) in-place on DVE (packed-bf16 fast mode) and
            # store from the Pool queue.
            gath_g = gaths[g]
            tot_g = statpool.tile([128, CH], F32, tag="tot")
            tots[g] = tot_g
            nc.vector.tensor_tensor(tot_g[:], gath_g[:, 0:CH],
                                    gath_g[:, CH:2 * CH], OP.add)
            for rr in range(2, N_CORES):
                nc.vector.tensor_tensor(tot_g[:], tot_g[:],
                                        gath_g[:, rr * CH:(rr + 1) * CH],
                                        OP.add)
            lng = lnpool.tile([128, CH], F32, tag="lng")
            nc.scalar.activation(lng[:], tot_g[:], AF.Ln)
            for j in range(CH):
                m = g * CH + j
                x_m = xs[g][j]
                nc.vector.tensor_scalar(x_m[:], x_m[:], lng[:, j:j + 1],
                                        None, OP.subtract)
                nc.gpsimd.dma_start(out_tgt[m * 128:(m + 1) * 128, :], x_m[:])

        for g in range(NCH):
            emit_chunk(g)
            if g >= 1:
                emit_output(g - 1)

        # ---- ext tail: scatter-add via one-hot matmul, overlaps last cc ----
        # batch pairs packed into 64-row psum halves; one-hots on DVE (int16
        # iota, 4x mode); Ln(scale=g1m) reads psum directly; clamp on DVE
        # (lagged one pair to avoid blocking the one-hot gen); DMA from Pool.
        ext_sts = [None] * NP

        def emit_ext_pair(p):
            ohs = []
            for bb in range(2):
                oh = scr.tile([S, VE], BF16, tag="scr")
                nc.vector.tensor_scalar(oh[:], iota_sb[:],
                                        idx_sb[:, 2 * p + bb:2 * p + bb + 1],
                                        None, OP.is_equal)
                ohs.append(oh)
            st = scr.tile([128, VE], BF16, tag="scr")
            ext_sts[p] = st
            for grp, gw in ((0, 4), (4, 4), (8, 2)):
                ps = pspool.tile([128, 2048], F32, tag="ps")
                for hl in range(2):
                    for bb in range(2):
                        b = 2 * p + bb
                        lhsT = attnT_sb[:, hl * RL + b * BP:
                                        hl * RL + (b + 1) * BP]
                        for q in range(gw):
                            nb = grp + q
                            nc.tensor.matmul(
                                ps[bb * BP:(bb + 1) * BP,
                                   q * 512: q * 512 + NW],
                                lhsT=lhsT,
                                rhs=ohs[bb][:, nb * NW:(nb + 1) * NW],
                                start=(hl == 0), stop=(hl == 1))
                src = ps[:].rearrange("p (b n) -> p b n", b=4)[:, :gw, :NW]
                dst = st[:, grp * NW:(grp + gw) * NW].rearrange(
                    "p (b n) -> p b n", b=gw)
                # ext = raw * g1m folded into Ln's per-partition scale;
                # clip is done in log space (Ln(0) = -inf clips to LOG_LO)
                nc.scalar.activation(dst, src, AF.Ln, scale=g1m[:, p:p + 1])

        def emit_ext_store(p):
            st = ext_sts[p]
            nc.vector.tensor_scalar(st[:], st[:], LOG_LO, LOG_HI,
                                    OP.max, OP.min)
            nc.gpsimd.dma_start(out_ext[2 * p], st[:T, :])
            nc.gpsimd.dma_start(out_ext[2 * p + 1], st[BP:BP + T, :])

        for p in range(NP):
            emit_ext_pair(p)
            if p >= 1:
                emit_ext_store(p - 1)
        emit_ext_store(NP - 1)

        # last chunk's output after the ext work so ACT/DVE stay busy while
        # the final collective is in flight.
        emit_output(NCH - 1)

    nc.compile()
    _dedupe_act_table_loads(nc)
    return nc


def _get_nc(with_bias=False):
    key = ("nc", with_bias)
    if key not in _CACHE:
        _CACHE[key] = _build(with_bias)
    return _CACHE[key]


def kernel(**inputs):
    hidden = np.asarray(inputs["hidden"], dtype=np.float32)
    dec = np.asarray(inputs["dec_rnn_output"], dtype=np.float32)
    attn = np.asarray(inputs["attn"], dtype=np.float32)
    c2e = np.asarray(inputs["copy_to_ext"])
    W = np.asarray(inputs["W"], dtype=np.float32)
    bvec = np.asarray(inputs["b"], dtype=np.float32)
    Wc = np.asarray(inputs["Wc"], dtype=np.float32)
    bc = np.asarray(inputs["bc"], dtype=np.float32)

    with_bias = bool(np.any(bvec))
    bf = ml_dtypes.bfloat16
    hT_np = np.ascontiguousarray(
        hidden.reshape(R, H).T.reshape(KT, 128, R)).astype(bf)
    wcT_np = np.ascontiguousarray(Wc.reshape(1, H).T.reshape(KT, 128, 1))
    bc_np = bc.reshape(1, 1)

    in_maps = []
    for c in range(N_CORES):
        vs = slice(c * VL, (c + 1) * VL)
        bs = slice(c * BL, (c + 1) * BL)
        wT_np = np.ascontiguousarray(W[vs].T.reshape(KT, 128, VL)).astype(bf)
        # dT cols are 64-padded b-major: col b*64 + t
        dpad = np.zeros((BL, BP, H), np.float32)
        dpad[:, :T] = dec[:, bs, :].transpose(1, 0, 2)
        dT_np = np.ascontiguousarray(
            dpad.reshape(RL, H).T.reshape(KT, 128, RL))
        # attnT[s, b*64 + t] = attn[t, c*BL+b, s]; hi/lo bf16 split
        apad = np.zeros((S, BL, BP), np.float32)
        apad[:, :, :T] = attn[:, bs, :].transpose(2, 1, 0)
        at = apad.reshape(S, RL)
        at_hi = at.astype(bf)
        at_lo = (at - at_hi.astype(np.float32)).astype(bf)
        attnT_np = np.ascontiguousarray(np.stack([at_hi, at_lo]))
        idx_np = np.ascontiguousarray(c2e[:, bs]).astype(np.float32)
        m = {"dT": dT_np, "wcT": wcT_np, "bc": bc_np,
             "attnT": attnT_np, "idx": idx_np}
        for k in range(KT):
            m[f"hT{k}"] = np.ascontiguousarray(hT_np[k])
            m[f"wT{k}"] = np.ascontiguousarray(wT_np[k])
        if with_bias:
            m["brow"] = bvec[vs].reshape(1, VL).astype(bf)
        in_maps.append(m)

    nc = _get_nc(with_bias)
    res = run_bass_kernel_spmd(nc, in_maps, core_ids=list(range(N_CORES)))

    out = np.empty((T, B, VT + VE), dtype=np.float32)
    for c in range(N_CORES):
        r = res.results[c]
        out[:, :, c * VL:(c + 1) * VL] = np.asarray(
            r["out_tgt"], dtype=np.float32).reshape(T, B, VL)
        out[:, c * BL:(c + 1) * BL, VT:] = np.asarray(
            r["out_ext"], dtype=np.float32).transpose(1, 0, 2)
    return out
